# revision 6
# baseline (speedup 1.0000x reference)
"""Trainium2 Bass kernel for nn_PolynomialRegressor.

Computes y = total_x @ W + b where total_x is all monomials of degree 1..3
over the 64 input features (47904 monomials), for x of shape [2048, 64].

Reformulation: augment x with a constant-1 feature (65 features).  Then the
whole computation (including bias and lower degrees) is one homogeneous
degree-3 form:

    y[b, o] = sum_{i<=j<=k<65} Wt[(i,j,k), o] * xt_i * xt_j * xt_k

Per-core algorithm (batch shard B=256, data parallel over 8 cores):
  1. z2[(i,j), b] = xt_i * xt_j for the L=2145 ordered pairs (i<=j), ordered
     by max index j then i ("prefix" ordering).  Built feature-major
     [pairs, B] in 17 chunks of 128 partitions: two PE selector matmuls
     expand xt rows into xi/xj operands, one DVE multiply forms the chunk.
  2. H3[(k,o), b] = sum_{pairs p with max<=k} U3[p, (k,o)] * z2[p, b].
     k's are grouped (<=12 per group so M=10*|g|<=128); each group is a
     PSUM-accumulated stack of chunk matmuls over the prefix of z2 chunks.
     U3 is W re-packed host-side into [128, Mg] blocks (zero-padded).
  3. y[o, b] = sum_k xt_k[b] * H3[(k,o), b]: expand xt via selector matmul,
     DVE multiply, then a small 0/1 matmul reduces k groups into y.

All matmuls use float32r (full fp32 data; 1 cycle/row on PE when N>=256).
"""

import numpy as np
from itertools import combinations_with_replacement

N_IN = 64
N_F = 65  # augmented features (x, 1)
N_OUT = 10
BATCH = 2048
N_CORES = 8
B = BATCH // N_CORES  # 256 rows per core

L = N_F * (N_F + 1) // 2  # 2145 pairs (i<=j)
NCHUNK = (L + 127) // 128  # 17
LPAD = NCHUNK * 128  # 2176

# T[k] = number of pairs with max index <= k
_T = [(k + 1) * (k + 2) // 2 for k in range(N_F)]

# contiguous k-groups (start, end), |group|<=12, minimizing total chunk-matmuls
GROUPS = [(0, 5), (5, 17), (17, 29), (29, 41), (41, 53), (53, 65)]
NG = len(GROUPS)
G_M = [10 * (e - s) for (s, e) in GROUPS]  # psum partition (M) per group
G_C = [(_T[e - 1] + 127) // 128 for (s, e) in GROUPS]  # z2 chunks per group
# u3 column layout: blocks ordered (g asc, chunk asc), each of width G_M[g]
G_COL0 = []
_c = 0
for g in range(NG):
    G_COL0.append(_c)
    _c += G_C[g] * G_M[g]
TOTCOL = _c  # 5090
E_COL0 = []  # ek column offset per group
_c = 0
for g in range(NG):
    E_COL0.append(_c)
    _c += G_M[g]
EKCOL = _c  # 650
MMAX = max(G_M)  # 120

_F32 = np.float32


def _pair_id(i, j):
    # pairs ordered: for j in 0..N_F-1: for i in 0..j
    return j * (j + 1) // 2 + i


def _static_tables():
    """Selector matrices ab/ek/sk (independent of runtime W)."""
    i_of = np.zeros(LPAD, np.int64)
    j_of = np.zeros(LPAD, np.int64)
    valid = np.zeros(LPAD, bool)
    for j in range(N_F):
        for i in range(j + 1):
            p = _pair_id(i, j)
            i_of[p] = i
            j_of[p] = j
            valid[p] = True

    # ab[:, c, 0:128] = A chunk c (xi selector), ab[:, c, 128:256] = B chunk c
    ab = np.zeros((N_F, NCHUNK, 256), _F32)
    p = np.arange(LPAD)
    c = p // 128
    r = p % 128
    ab[i_of[valid], c[valid], r[valid]] = 1.0
    ab[j_of[valid], c[valid], 128 + r[valid]] = 1.0

    # ek[f, E_COL0[g] + (k-k0)*10 + o] = 1 iff f == k
    ek = np.zeros((N_F, EKCOL), _F32)
    # sk[(k-k0)*10 + o, 10*g + o] = 1
    sk = np.zeros((MMAX, 10 * NG), _F32)
    for g, (k0, k1) in enumerate(GROUPS):
        for k in range(k0, k1):
            for o in range(N_OUT):
                ek[k, E_COL0[g] + (k - k0) * 10 + o] = 1.0
                sk[(k - k0) * 10 + o, 10 * g + o] = 1.0
    return ab, ek, sk


def _monomial_index_arrays():
    """(pair_id, k) for every row of the reference W, plus the bias row."""
    rows_p = []
    rows_k = []
    # degree 1: (i,) -> (i, 64, 64)
    i1 = np.arange(N_IN)
    rows_p.append(_pair_id(i1, np.full(N_IN, N_IN)))
    rows_k.append(np.full(N_IN, N_IN))
    # degree 2: (i,j) -> (i, j, 64)
    d2 = np.array(list(combinations_with_replacement(range(N_IN), 2)), np.int64)
    rows_p.append(_pair_id(d2[:, 0], d2[:, 1]))
    rows_k.append(np.full(len(d2), N_IN))
    # degree 3: (i,j,k)
    d3 = np.array(list(combinations_with_replacement(range(N_IN), 3)), np.int64)
    rows_p.append(_pair_id(d3[:, 0], d3[:, 1]))
    rows_k.append(d3[:, 2])
    # bias -> (64,64,64)
    rows_p.append(np.array([_pair_id(N_IN, N_IN)]))
    rows_k.append(np.array([N_IN]))
    return np.concatenate(rows_p), np.concatenate(rows_k)


_IDX_CACHE = None


def _pack_u3(W, b):
    """Pack [47904,10] W + [10] b into the u3 block matrix [128, TOTCOL]."""
    global _IDX_CACHE
    if _IDX_CACHE is None:
        pid, kk = _monomial_index_arrays()
        group_of_k = np.zeros(N_F, np.int64)
        k0_of_g = np.zeros(NG, np.int64)
        for g, (k0, k1) in enumerate(GROUPS):
            group_of_k[k0:k1] = g
            k0_of_g[g] = k0
        g = group_of_k[kk]
        chunk = pid // 128
        row = pid % 128
        col0 = np.array(G_COL0)[g] + chunk * np.array(G_M)[g]
        col = col0 + (kk - k0_of_g[g]) * N_OUT
        _IDX_CACHE = (row, col)
    row, col = _IDX_CACHE
    Wfull = np.concatenate([np.asarray(W, _F32), np.asarray(b, _F32)[None, :]], axis=0)
    u3 = np.zeros((128, TOTCOL), _F32)
    u3[row[:, None], col[:, None] + np.arange(N_OUT)[None, :]] = Wfull
    return u3


_PROGRAM = None


def _build_program():
    import concourse.mybir as mybir
    import concourse.tile as tile
    from concourse import bacc

    f32 = mybir.dt.float32
    f32r = mybir.dt.float32r

    nc = bacc.Bacc(None, target_bir_lowering=False)
    xt_d = nc.dram_tensor("xt", [N_F, B], f32r, kind="ExternalInput")
    ab_d = nc.dram_tensor("ab", [N_F, NCHUNK, 256], f32r, kind="ExternalInput")
    u3_d = nc.dram_tensor("u3", [128, TOTCOL], f32r, kind="ExternalInput")
    ek_d = nc.dram_tensor("ek", [N_F, EKCOL], f32r, kind="ExternalInput")
    sk_d = nc.dram_tensor("sk", [MMAX, 10 * NG], f32r, kind="ExternalInput")
    y_d = nc.dram_tensor("y", [N_OUT, B], f32, kind="ExternalOutput")

    with tile.TileContext(nc) as tc:
        with (
            tc.tile_pool(name="const", bufs=1) as const,
            tc.tile_pool(name="work", bufs=3) as work,
            tc.tile_pool(name="m3p", bufs=2) as m3p,
            tc.tile_pool(name="psz", bufs=2, space="PSUM") as psz,
            tc.tile_pool(name="psh", bufs=2, space="PSUM") as psh,
            tc.tile_pool(name="psk", bufs=2, space="PSUM") as psk,
            tc.tile_pool(name="psy", bufs=1, space="PSUM") as psy,
        ):
            xt_sb = const.tile([N_F, B], f32r)
            nc.sync.dma_start(out=xt_sb[:], in_=xt_d[:])
            ek_sb = const.tile([N_F, EKCOL], f32r)
            nc.sync.dma_start(out=ek_sb[:], in_=ek_d[:])
            sk_sb = const.tile([MMAX, 10 * NG], f32r)
            nc.sync.dma_start(out=sk_sb[:], in_=sk_d[:])
            ab_sb = []
            for c in range(NCHUNK):
                t = const.tile([N_F, 256], f32r, tag=f"ab{c}")
                nc.sync.dma_start(out=t[:], in_=ab_d[:, c, :])
                ab_sb.append(t)
            u3_sb = []
            for g in range(NG):
                lo = G_COL0[g]
                hi = lo + G_C[g] * G_M[g]
                t = const.tile([128, hi - lo], f32r, tag=f"u3{g}")
                nc.sync.dma_start(out=t[:], in_=u3_d[:, lo:hi])
                u3_sb.append(t)

            # stage 1: z2 chunks
            z2_sb = const.tile([128, NCHUNK, B], f32r)
            for c in range(NCHUNK):
                ps = psz.tile([128, 2 * B], f32)
                nc.tensor.matmul(
                    ps[:, 0:B],
                    ab_sb[c][:, 0:128],
                    xt_sb[:],
                    start=True,
                    stop=True,
                )
                nc.tensor.matmul(
                    ps[:, B : 2 * B],
                    ab_sb[c][:, 128:256],
                    xt_sb[:],
                    start=True,
                    stop=True,
                )
                xi_sb = work.tile([128, B], f32)
                nc.scalar.copy(out=xi_sb[:], in_=ps[:, 0:B])
                nc.vector.tensor_mul(z2_sb[:, c, :], xi_sb[:], ps[:, B : 2 * B])

            # stages 2+3: per-group prefix matmuls, xk multiply, reduction
            y_ps = psy.tile([N_OUT, B], f32)
            for g in range(NG):
                Mg = G_M[g]
                Cg = G_C[g]
                h_ps = psh.tile([Mg, B], f32)
                for ci in range(Cg):
                    lo = ci * Mg
                    nc.tensor.matmul(
                        h_ps[:],
                        u3_sb[g][:, lo : lo + Mg],
                        z2_sb[:, ci, :],
                        start=(ci == 0),
                        stop=(ci == Cg - 1),
                    )
                xk_ps = psk.tile([Mg, B], f32)
                nc.tensor.matmul(
                    xk_ps[:],
                    ek_sb[:, E_COL0[g] : E_COL0[g] + Mg],
                    xt_sb[:],
                    start=True,
                    stop=True,
                )
                xk_sb = work.tile([Mg, B], f32, tag="xk")
                nc.scalar.copy(out=xk_sb[:], in_=xk_ps[:])
                m3_sb = m3p.tile([Mg, B], f32r)
                nc.vector.tensor_mul(m3_sb[:], xk_sb[:], h_ps[:])
                nc.tensor.matmul(
                    y_ps[:],
                    sk_sb[0:Mg, 10 * g : 10 * (g + 1)],
                    m3_sb[:],
                    start=(g == 0),
                    stop=(g == NG - 1),
                )

            y_sb = const.tile([N_OUT, B], f32)
            nc.scalar.copy(out=y_sb[:], in_=y_ps[:])
            nc.sync.dma_start(out=y_d[:], in_=y_sb[:])

    nc.compile()
    return nc


def _get_program():
    global _PROGRAM
    if _PROGRAM is None:
        _PROGRAM = _build_program()
    return _PROGRAM


def make_in_maps(x, W, b):
    """Host-side input marshaling: shard x by batch, pack W, replicate tables."""
    x = np.asarray(x, _F32)
    ab, ek, sk = _static_tables()
    u3 = _pack_u3(W, b)
    in_maps = []
    for cid in range(N_CORES):
        xs = x[cid * B : (cid + 1) * B]  # [B, 64]
        xt = np.empty((N_F, B), _F32)
        xt[:N_IN] = xs.T
        xt[N_IN] = 1.0
        in_maps.append({"xt": xt, "ab": ab, "u3": u3, "ek": ek, "sk": sk})
    return in_maps


def kernel(x, W, b):
    from concourse.bass_utils import run_bass_kernel_spmd

    nc = _get_program()
    in_maps = make_in_maps(x, W, b)
    res = run_bass_kernel_spmd(nc, in_maps, core_ids=list(range(N_CORES)))
    y = np.concatenate([r["y"] for r in res.results], axis=1)  # [10, 2048]
    return np.ascontiguousarray(y.T)
